# revision 2
# baseline (speedup 1.0000x reference)
"""ActiveConv Trainium2 kernel.

out[b,o,y,x] = sum_c conv_w[o,c] * bilinear_displace(repeat(inp,4)[b,c], offsets[c]) + conv_b[o]

Key observations:
  * The per-channel displacement is a constant (dx, dy) per channel -> a uniform
    integer shift (ix, iy) plus a bilinear blend with constant corner weights
    (1-fy|fy) x (1-fx|fx).
  * Folding the corner weights into conv_w host-side gives 4 weight matrices
    W_st[o,c] = conv_w[o,c] * wy_s[c] * wx_t[c]; the device computation becomes
      out[o, n] = sum_{s,t} W_st @ G[:, n + s*Wp + t]
    i.e. 4 shifted matmul-accumulate passes over a single gathered tensor G,
    where G[c] is the input channel c//4 as a zero-padded image shifted by the
    channel's integer offset. G is built with per-partition element-granular
    indirect DMA gathers (int32 indices, one descriptor per channel).
  * Data-parallel over batch: 16 batches / 8 cores = 2 per core. No collectives.

Device pipeline per core (2 batches):
  gather G (4 indirect DMAs, [128, EG] bf16 each)  ->  bf16 matmuls (K=128
  chunks x 4 corners, N=512 pixel tiles) accumulating f32 in PSUM  ->  DVE
  bias-add PSUM->SBUF  ->  DMA out f32.
"""

import numpy as np
import ml_dtypes

B, C_IN, H, W = 16, 64, 64, 64
OPC = 4
C = C_IN * OPC          # 256
C_OUT = 128
NCORES = 8
BPC = B // NCORES       # batches per core
HW = H * W

_PLAN_CACHE = {}


def _build_plan(Hp, Wp, EG, V):
    """Trace + schedule the bass kernel for given padded-image geometry."""
    import concourse.bacc as bacc
    import concourse.bass as bass
    import concourse.tile as tile
    import concourse.mybir as mybir

    nc = bacc.Bacc(None, target_bir_lowering=False)

    pb = nc.dram_tensor("pb", [V, 1], mybir.dt.bfloat16, kind="ExternalInput")
    gidx = nc.dram_tensor("gidx", [128, 4], mybir.dt.int32, kind="ExternalInput")
    wts = nc.dram_tensor("wts", [128, 8 * 128], mybir.dt.bfloat16, kind="ExternalInput")
    bias = nc.dram_tensor("bias", [128, 1], mybir.dt.float32, kind="ExternalInput")
    out = nc.dram_tensor("out", [BPC, 128, HW], mybir.dt.float32, kind="ExternalOutput")

    NYT = HW // (8 * W)  # pixel tiles of 512 = 8 image rows each

    with tile.TileContext(nc) as tc:
        with (
            tc.tile_pool(name="const", bufs=1) as const,
            tc.tile_pool(name="gpool", bufs=1) as gpool,
            tc.tile_pool(name="psum", bufs=4, space="PSUM") as psum,
            tc.tile_pool(name="outp", bufs=4) as outp,
        ):
            idx_t = const.tile([128, 4], mybir.dt.int32)
            nc.sync.dma_start(out=idx_t[:], in_=gidx[:])
            wts_t = const.tile([128, 8 * 128], mybir.dt.bfloat16)
            nc.sync.dma_start(out=wts_t[:], in_=wts[:])
            bias_t = const.tile([128, 1], mybir.dt.float32)
            nc.sync.dma_start(out=bias_t[:], in_=bias[:])

            # G gather: chunk q = b*2 + h holds channels h*128..h*128+127 of
            # batch b, shifted per channel. One indirect DMA each.
            g = [gpool.tile([128, EG], mybir.dt.bfloat16, name=f"g{q}") for q in range(4)]
            for q in range(4):
                nc.gpsimd.indirect_dma_start(
                    out=g[q][:],
                    out_offset=None,
                    in_=pb[:],
                    in_offset=bass.IndirectOffsetOnAxis(ap=idx_t[:, q : q + 1], axis=0),
                )

            for b in range(BPC):
                for yt in range(NYT):
                    pt = psum.tile([128, 512], mybir.dt.float32)
                    k = 0
                    for h in range(2):
                        q = b * 2 + h
                        for s in range(2):
                            for t in range(2):
                                base = (8 * yt + s) * Wp + t
                                rhs = (
                                    g[q][:, base : base + 8 * Wp]
                                    .rearrange("p (r w) -> p r w", w=Wp)[:, :, 0:W]
                                )
                                nc.tensor.matmul(
                                    pt[:],
                                    lhsT=wts_t[:, (h * 4 + s * 2 + t) * 128 : (h * 4 + s * 2 + t + 1) * 128],
                                    rhs=rhs,
                                    start=(k == 0),
                                    stop=(k == 7),
                                )
                                k += 1
                    ot = outp.tile([128, 512], mybir.dt.float32)
                    nc.vector.tensor_scalar_add(ot[:], pt[:], bias_t[:, :1])
                    nc.sync.dma_start(
                        out=out[b, :, yt * 512 : (yt + 1) * 512], in_=ot[:]
                    )

    nc.finalize()
    return nc


def _prep(offsets, conv_w, conv_b):
    """Host-side folding of displacement into gather indices + weights."""
    dx = offsets[:, 0].astype(np.float64)
    dy = offsets[:, 1].astype(np.float64)
    ix = np.floor(dx).astype(np.int64)
    iy = np.floor(dy).astype(np.int64)
    fx = (dx - ix).astype(np.float32)
    fy = (dy - iy).astype(np.float32)

    # channels whose shifted window cannot overlap the image contribute zero
    alive = (iy > -(H + 1)) & (iy < H) & (ix > -(W + 1)) & (ix < W)
    ix = np.where(alive, ix, 0)
    iy = np.where(alive, iy, 0)

    px0 = max(0, -int(ix.min()))
    px1 = max(0, int(ix.max()) + 2)
    py0 = max(0, -int(iy.min()))
    py1 = max(0, int(iy.max()) + 2)
    Hp, Wp = H + py0 + py1, W + px0 + px1

    # corner-folded weight matrices, lhsT layout [k, (h*4 + s*2 + t)*128 + m]
    w = conv_w.astype(np.float32)
    wy = [(1.0 - fy), fy]
    wx = [(1.0 - fx), fx]
    wts = np.zeros((128, 8 * 128), dtype=np.float32)
    for h in range(2):
        cs = slice(h * 128, (h + 1) * 128)
        for s in range(2):
            for t in range(2):
                m = (w[:, cs] * (wy[s][cs] * wx[t][cs] * alive[cs])[None, :]).astype(
                    np.float32
                )  # [o, k]
                wts[:, (h * 4 + s * 2 + t) * 128 : (h * 4 + s * 2 + t + 1) * 128] = m.T
    wts = wts.astype(ml_dtypes.bfloat16)

    EG = 66 * Wp
    FP = Hp * Wp
    V = BPC * C_IN * FP + EG  # + tail slack so idx+EG stays in bounds

    gidx = np.zeros((128, 4), dtype=np.int32)
    for q in range(4):
        b, h = q // 2, q % 2
        for p in range(128):
            c = h * 128 + p
            cin = c // 4
            gidx[p, q] = (b * C_IN + cin) * FP + (py0 + iy[c]) * Wp + px0 + ix[c]
    assert gidx.min() >= 0 and gidx.max() + EG <= V

    bias = conv_b.astype(np.float32).reshape(128, 1)
    return dict(
        px0=px0, py0=py0, Hp=Hp, Wp=Wp, EG=EG, V=V,
        wts=wts, gidx=gidx, bias=bias,
    )


def kernel(inp, offsets, conv_w, conv_b, _trace=False):
    import concourse.bass_utils as bu

    inp = np.asarray(inp)
    offsets = np.asarray(offsets)
    conv_w = np.asarray(conv_w)
    conv_b = np.asarray(conv_b)

    p = _prep(offsets, conv_w, conv_b)
    Hp, Wp, EG, V = p["Hp"], p["Wp"], p["EG"], p["V"]

    key = (Hp, Wp)
    if key not in _PLAN_CACHE:
        _PLAN_CACHE[key] = _build_plan(Hp, Wp, EG, V)
    nc = _PLAN_CACHE[key]

    # host-padded bf16 images: [B, C_IN, Hp, Wp]
    pbf = np.zeros((B, C_IN, Hp, Wp), dtype=ml_dtypes.bfloat16)
    pbf[:, :, p["py0"] : p["py0"] + H, p["px0"] : p["px0"] + W] = inp.astype(
        ml_dtypes.bfloat16
    )

    tail = np.zeros(EG, dtype=ml_dtypes.bfloat16)
    in_maps = []
    for core in range(NCORES):
        pb_core = np.concatenate(
            [pbf[core * BPC : (core + 1) * BPC].reshape(-1), tail]
        ).reshape(V, 1)
        in_maps.append(
            {"pb": pb_core, "gidx": p["gidx"], "wts": p["wts"], "bias": p["bias"]}
        )

    res = bu.run_bass_kernel_spmd(
        nc, in_maps, core_ids=list(range(NCORES)), trace=_trace
    )
    if _trace:
        kernel.last_exec_ns = res.exec_time_ns
        kernel.last_mean_exec_ns = res.mean_exec_time_ns
        it = res.instructions_and_trace
        kernel.last_trace_path = it[1] if it else None

    out = np.concatenate(
        [res.results[i]["out"].reshape(BPC, C_OUT, H, W) for i in range(NCORES)],
        axis=0,
    )
    return out

